# revision 1
# baseline (speedup 1.0000x reference)
"""Sliding-window MHA Trainium2 kernel, sharded over 8 NeuronCores.

Problem (hardcoded): B=2, L=2048, D=1024, H=16 heads (hd=64), window
|i-j| <= 256, fp32 I/O.

Sharding: core = b*4 + g  (b in 0..1 batches, g in 0..3 head-groups of 4
heads). Each core computes QKV projection for its 4 heads, banded
attention, and a partial output projection (its 256 columns of the head
concat). Host sums the 4 partial projections per batch and adds out_b.

Device pipeline per core (matmuls fp32r, E/V fp16):
  x^T (host-pretransposed) -> Q^T,K^T [f,t] and V [t,f] projections
  per head: per k-block S^T = K^T.T Q^T over the 640-wide band window,
  E^T = exp(S/8) fp16 with static triangle masks on edge strips,
  O [q,65] = E^T-chunks.T @ [V|1] accumulated in PSUM, normalized by
  col 64, PE-transposed to O'^T, projected with the out_w slice.
  All phases are software-pipelined in emission order so PE (matmul),
  ACT (exp), DVE (copies) and DMA overlap.
"""

import numpy as np

import concourse.bacc as bacc
import concourse.mybir as mybir
import concourse.tile as tile
from concourse.bass_utils import run_bass_kernel_spmd
from concourse.masks import make_identity

F32 = mybir.dt.float32
F32R = mybir.dt.float32r
F16 = mybir.dt.float16

P = 128
L = 2048
D = 1024
NH = 4          # heads per core
HD = 64
FQK = 512       # q+k feature rows per core (4 heads * 64 * 2)
FV = 256        # v feature rows per core
WIN = 256
KB = L // P     # 16 k-blocks
N_CORES = 8


def _window(kb):
    """q-range [qlo, qhi) covered by k-block kb under |q-k| <= WIN."""
    k0 = kb * P
    qlo = max(0, k0 - WIN)
    qhi = min(L, k0 + P + WIN)
    return qlo, qhi - qlo


def _build_nc():
    nc = bacc.Bacc(
        "TRN2", target_bir_lowering=False, debug=False, num_devices=N_CORES
    )
    xt_d = nc.dram_tensor("xt", [D, L], F16, kind="ExternalInput").ap()
    wqk_d = nc.dram_tensor("wqk_t", [4, D, P], F16, kind="ExternalInput").ap()
    wv_d = nc.dram_tensor("wv_t", [D, FV], F16, kind="ExternalInput").ap()
    wo_d = nc.dram_tensor("wo_t", [FV, D], F32R, kind="ExternalInput").ap()
    bqk_d = nc.dram_tensor("bqk", [P, 4], F32, kind="ExternalInput").ap()
    bv_d = nc.dram_tensor("bv", [1, FV], F32, kind="ExternalInput").ap()
    y_d = nc.dram_tensor("y", [L, D], F16, kind="ExternalOutput").ap()

    with tile.TileContext(nc) as tc:
        _emit(nc, tc, xt_d, wqk_d, wv_d, wo_d, bqk_d, bv_d, y_d)
    nc.compile()
    return nc


def _emit(nc, tc, xt_d, wqk_d, wv_d, wo_d, bqk_d, bv_d, y_d):
    import contextlib

    ctx = contextlib.ExitStack()
    with ctx:
        const = ctx.enter_context(tc.tile_pool(name="const", bufs=1))
        w_pool = ctx.enter_context(tc.tile_pool(name="w", bufs=1))
        qk_pool = ctx.enter_context(tc.tile_pool(name="qk", bufs=1))
        v_pool = ctx.enter_context(tc.tile_pool(name="v", bufs=1))
        xt_pool = ctx.enter_context(tc.tile_pool(name="xt", bufs=1))
        e_pool = ctx.enter_context(tc.tile_pool(name="e", bufs=9))
        oT_pool = ctx.enter_context(tc.tile_pool(name="oT", bufs=1))
        sm_pool = ctx.enter_context(tc.tile_pool(name="sm", bufs=3))
        ysb_pool = ctx.enter_context(tc.tile_pool(name="ysb", bufs=4))
        opr_pool = ctx.enter_context(tc.tile_pool(name="opr", bufs=20))
        ppool = ctx.enter_context(tc.tile_pool(name="ppsum", bufs=2, space="PSUM"))

        # ---- weight/bias/x DMAs (fine-grained, pipeline-ordered) --------
        wqk_sb = w_pool.tile([P, 4, 8, P], F16)
        wv_sb = w_pool.tile([P, 8, FV], F16)
        wo_sb = w_pool.tile([P, 2, D], F32R)
        xt_sb = xt_pool.tile([P, 8, L], F16)
        wqk_re = wqk_d.rearrange("f (c p) n -> p f c n", p=P)
        xt_re = xt_d.rearrange("(c p) t -> p c t", p=P)

        bqk_sb = const.tile([P, 4], F32)
        nc.sync.dma_start(bqk_sb[:], bqk_d[:])
        bv_row = const.tile([1, FV], F32)
        nc.sync.dma_start(bv_row[:], bv_d[:])

        # first compute wave needs wqk fc0/fc2 + xt t-slice 0
        nc.sync.dma_start(wqk_sb[:, 0, :, :], wqk_re[:, 0, :, :])
        for dc in range(8):
            nc.sync.dma_start(xt_sb[:, dc, 0:512], xt_re[:, dc, 0:512])
        nc.sync.dma_start(wqk_sb[:, 2, :, :], wqk_re[:, 2, :, :])
        nc.sync.dma_start(wv_sb[:], wv_d.rearrange("(c p) n -> p c n", p=P))
        for t in range(1, 4):
            for dc in range(8):
                nc.sync.dma_start(
                    xt_sb[:, dc, t * 512:(t + 1) * 512],
                    xt_re[:, dc, t * 512:(t + 1) * 512])
        for fc in (1, 3):
            nc.sync.dma_start(wqk_sb[:, fc, :, :], wqk_re[:, fc, :, :])
        nc.sync.dma_start(wo_sb[:], wo_d.rearrange("(c p) n -> p c n", p=P))

        # ---- constants --------------------------------------------------
        ident_f32 = const.tile([P, P], F32)
        make_identity(nc, ident_f32[:])
        ident = const.tile([P, P], F32R)
        nc.vector.tensor_copy(ident[:], ident_f32[:])
        # mask_l[kl, c] = 1 if c >= kl else 0   (upper tri incl diag)
        mask_l = const.tile([P, P], F16)
        nc.gpsimd.memset(mask_l[:], 1.0)
        nc.gpsimd.affine_select(
            out=mask_l[:], in_=mask_l[:],
            compare_op=mybir.AluOpType.is_ge, fill=0.0,
            base=0, pattern=[[1, P]], channel_multiplier=-1,
        )
        # mask_r[kl, c] = 1 if c <= kl else 0   (lower tri incl diag)
        mask_r = const.tile([P, P], F16)
        nc.gpsimd.memset(mask_r[:], 1.0)
        nc.gpsimd.affine_select(
            out=mask_r[:], in_=mask_r[:],
            compare_op=mybir.AluOpType.is_ge, fill=0.0,
            base=0, pattern=[[-1, P]], channel_multiplier=1,
        )
        bv_bc = const.tile([P, FV], F32)
        nc.gpsimd.partition_broadcast(bv_bc[:], bv_row[:])

        # Q^T / K^T per head-pair chunk: [f%128, chunk, t]
        qT = qk_pool.tile([P, 2, L], F32R)
        kT = qk_pool.tile([P, 2, L], F32R)
        # V per k-block, heads side by side, each with ones col (rowsum)
        v_ext = v_pool.tile([P, KB, NH * (HD + 1)], F16)
        nc.gpsimd.memset(v_ext[:], 1.0)
        oT = oT_pool.tile([P, 2, L], F32R)
        pair_store = {}

        # ---- emission helpers -------------------------------------------
        def qk_proj(fc, t):
            pq = ppool.tile([P, 512], F32, tag="pqk", name="pq")
            for dc in range(8):
                nc.tensor.matmul(
                    pq[:],
                    lhsT=wqk_sb[:, fc, dc, :],
                    rhs=xt_sb[:, dc, t * 512:(t + 1) * 512],
                    start=(dc == 0), stop=(dc == 7),
                )
            dest = qT if fc < 2 else kT
            nc.vector.tensor_scalar_add(
                dest[:, fc % 2, t * 512:(t + 1) * 512], pq[:],
                bqk_sb[:, fc:fc + 1],
            )

        def v_proj(t):
            pv = ppool.tile([P, 512], F32, tag="pqk", name="pv")
            for dc in range(8):
                nc.tensor.matmul(
                    pv[:, 0:FV],
                    lhsT=xt_sb[:, dc, t * P:(t + 1) * P],
                    rhs=wv_sb[:, dc, :],
                    start=(dc == 0), stop=(dc == 7),
                )
            nc.vector.tensor_add(
                v_ext[:, t, :].rearrange("p (h c) -> p h c", h=NH)[:, :, 0:HD],
                pv[:, 0:FV].rearrange("p (h c) -> p h c", h=NH),
                bv_bc[:].rearrange("p (h c) -> p h c", h=NH),
            )

        def phase_b(h, kb, e_tiles):
            cc, po = h // 2, (h % 2) * HD
            qlo, w = _window(kb)
            e_sb = e_pool.tile([P, 640], F16, tag="e", name="e_sb")
            e_tiles[kb] = e_sb
            s_ps = ppool.tile([P, 1024], F32, tag="s", name="s_ps")
            if w == 640:
                pieces = [(0, 0, 320), (320, 512, 320)]
            else:
                pieces = [(0, 0, w)]
            for qoff, poff, pw in pieces:
                nc.tensor.matmul(
                    s_ps[:, poff:poff + pw],
                    lhsT=kT[po:po + HD, cc, kb * P:(kb + 1) * P],
                    rhs=qT[po:po + HD, cc, qlo + qoff:qlo + qoff + pw],
                    start=True, stop=True,
                )
            if w == 640:
                src = s_ps[:].rearrange("p (g c) -> p g c", g=2)[:, :, 0:320]
                dst = e_sb[:].rearrange("p (g c) -> p g c", g=2)
            else:
                src = s_ps[:, 0:w]
                dst = e_sb[0:P, 0:w]
            nc.scalar.activation(
                dst, src, mybir.ActivationFunctionType.Exp, scale=0.125)
            if kb >= 2:
                nc.gpsimd.tensor_mul(e_sb[:, 0:P], e_sb[:, 0:P], mask_l[:])
            if kb <= KB - 3:
                nc.gpsimd.tensor_mul(
                    e_sb[:, w - P:w], e_sb[:, w - P:w], mask_r[:])

        def phase_c1(h, qt, e_tiles, store):
            cc, po = h // 2, (h % 2) * HD
            kbs = range(max(0, qt - 2), min(KB, qt + 3))
            ot = ppool.tile([P, 512], F32, tag="ot", name="ot")
            o_ps = ot[:, 0:HD + 1]
            for i, kb in enumerate(kbs):
                qlo, w = _window(kb)
                off = qt * P - qlo
                nc.tensor.matmul(
                    o_ps,
                    lhsT=e_tiles[kb][:, off:off + P],
                    rhs=v_ext[:, kb, h * 65:h * 65 + 65],
                    start=(i == 0), stop=(i == len(kbs) - 1),
                )
            rr = sm_pool.tile([P, 1], F32, tag="rr", name="rr")
            nc.vector.reciprocal(rr[:], o_ps[:, HD:HD + 1])
            # normalized head-output into its half of the pair tile [128,128]
            if h % 2 == 0:
                opr = opr_pool.tile([P, P], F32R, tag="opr", name="opr")
                pair_store[qt] = opr
            else:
                opr = pair_store[qt]
            nc.vector.tensor_scalar_mul(
                opr[:, po:po + HD], o_ps[:, 0:HD], rr[:])
            store[qt] = ot

        def phase_c2(h, qt, store):
            # h odd: transpose the completed [128,128] head-pair tile
            cc = h // 2
            ot = store.pop(qt)
            opr = pair_store.pop(qt)
            t_ps = ot[:, 128:256].bitcast(F32R)
            nc.tensor.transpose(t_ps, opr[:], ident[:])
            nc.vector.tensor_copy(oT[:, cc, qt * P:(qt + 1) * P], t_ps)

        def phase_d(qt):
            y_sb = ysb_pool.tile([P, D], F16, tag="ysb", name="y_sb")
            y_ps = [ppool.tile([P, 512], F32, tag="pqk", name=f"y_ps{i}")
                    for i in range(2)]
            for cc in range(2):
                for half in range(2):
                    nc.tensor.matmul(
                        y_ps[half][:],
                        lhsT=oT[:, cc, qt * P:(qt + 1) * P],
                        rhs=wo_sb[:, cc, half * 512:(half + 1) * 512],
                        start=(cc == 0), stop=(cc == 1),
                    )
            for half in range(2):
                sl = slice(half * 512, (half + 1) * 512)
                if half == 0:
                    nc.vector.tensor_copy(y_sb[:, sl], y_ps[half][:])
                else:
                    nc.scalar.copy(y_sb[:, sl], y_ps[half][:])
            nc.sync.dma_start(y_d[qt * P:(qt + 1) * P, :], y_sb[:])

        # ---- pipelined emission ----------------------------------------
        # per-head step loop: B(kb) leads; C1 lags 4 (exp+mask slack),
        # C2 lags 5, D lags 6 (h3 only). Projections stream in as waves.
        C1_LAG, C2_LAG, D_LAG = 4, 5, 6

        def run_head(h, inject=None):
            e_tiles, store = {}, {}
            for kb in range(KB):
                phase_b(h, kb, e_tiles)
                if inject:
                    for f in inject.get(kb, ()):
                        f()
                if kb >= C1_LAG:
                    phase_c1(h, kb - C1_LAG, e_tiles, store)
                if h % 2 == 1 and kb >= C2_LAG:
                    phase_c2(h, kb - C2_LAG, store)
                if h == NH - 1 and kb >= D_LAG:
                    phase_d(kb - D_LAG)
            for qt in range(KB - C1_LAG, KB):
                phase_c1(h, qt, e_tiles, store)
                if h % 2 == 1:
                    phase_c2(h, qt - 1, store)
                    if h == NH - 1:
                        phase_d(qt - 2)
            if h % 2 == 1:
                phase_c2(h, KB - 1, store)
            if h == NH - 1:
                phase_d(KB - 2)
                phase_d(KB - 1)

        # h0: interleave qk (fc0/fc2) waves + all V projections.
        # B(h0,kb) needs qT cols up to kb*128+384 -> t-wave (kb+2)//4.
        inj0 = {}
        emitted_t = [0]
        qk_proj(0, 0)
        qk_proj(2, 0)
        v_proj(0)
        v_proj(1)
        for kb in range(KB):
            items = []
            t_need = min(3, (kb + 3 + 2) // 4)   # one wave ahead of need
            while emitted_t[0] < t_need:
                emitted_t[0] += 1
                tt = emitted_t[0]
                items.append(lambda tt=tt: qk_proj(0, tt))
                items.append(lambda tt=tt: qk_proj(2, tt))
            if kb + 2 < KB:
                items.append(lambda kb=kb: v_proj(kb + 2))
            inj0[kb] = items
        run_head(0, inj0)

        # fc1/fc3 projections (needed by h2/h3) split across h1 and h2:
        # h2's B(kb) needs wave t=(kb+2)//4, so t2/t3 tiles stream in-head.
        inj1 = {}
        for i, (fc, t) in enumerate([(1, 0), (3, 0), (1, 1), (3, 1)]):
            inj1.setdefault(3 * i, []).append(lambda fc=fc, t=t: qk_proj(fc, t))
        run_head(1, inj1)
        inj2 = {}
        for step, (fc, t) in zip((0, 2, 5, 7), [(1, 2), (3, 2), (1, 3), (3, 3)]):
            inj2.setdefault(step, []).append(lambda fc=fc, t=t: qk_proj(fc, t))
        run_head(2, inj2)
        run_head(3)


_NC_CACHE = None


def _get_nc():
    global _NC_CACHE
    if _NC_CACHE is None:
        _NC_CACHE = _build_nc()
    return _NC_CACHE


def kernel(x, qkv_w, qkv_b, out_w, out_b):
    x = np.asarray(x, dtype=np.float32)
    qkv_w = np.asarray(qkv_w, dtype=np.float32)
    qkv_b = np.asarray(qkv_b, dtype=np.float32)
    out_w = np.asarray(out_w, dtype=np.float32)
    out_b = np.asarray(out_b, dtype=np.float32)
    B = x.shape[0]
    assert x.shape == (B, L, D) and B * 4 == N_CORES

    nc = _get_nc()

    xts = [np.ascontiguousarray(x[b].T.astype(np.float16)) for b in range(B)]
    in_maps = []
    for core in range(N_CORES):
        b, g = divmod(core, 4)
        rq = slice(g * FV, (g + 1) * FV)
        rk = slice(D + g * FV, D + (g + 1) * FV)
        rv = slice(2 * D + g * FV, 2 * D + (g + 1) * FV)
        wqk_t = np.ascontiguousarray(
            np.concatenate([qkv_w[rq], qkv_w[rk]], axis=0).T)      # [D, 512]
        wqk_fc = np.ascontiguousarray(
            wqk_t.reshape(D, 4, P).transpose(1, 0, 2).astype(np.float16))
        wv_t = np.ascontiguousarray(qkv_w[rv].T.astype(np.float16))
        wo_t = np.ascontiguousarray(out_w[:, g * FV:(g + 1) * FV].T)
        bqk = np.ascontiguousarray(
            np.concatenate([qkv_b[rq], qkv_b[rk]]).reshape(4, P).T)
        bv = np.ascontiguousarray(qkv_b[rv].reshape(1, FV))
        in_maps.append({
            "xt": xts[b], "wqk_t": wqk_fc, "wv_t": wv_t, "wo_t": wo_t,
            "bqk": bqk, "bv": bv,
        })

    res = run_bass_kernel_spmd(nc, in_maps, list(range(N_CORES)))
    y = np.empty((B, L, D), dtype=np.float32)
    for b in range(B):
        acc = res.results[b * 4 + 0]["y"].astype(np.float32)
        for g in range(1, 4):
            acc = acc + res.results[b * 4 + g]["y"]
        y[b] = acc
    if np.any(out_b):
        y += out_b
    return y



# revision 12
# speedup vs baseline: 1.1013x; 1.1013x over previous
"""Sliding-window MHA Trainium2 kernel, sharded over 8 NeuronCores.

Problem (hardcoded): B=2, L=2048, D=1024, H=16 heads (hd=64), window
|i-j| <= 256, fp32 I/O.

Sharding: core = b*4 + g (b in 0..1 batches, g in 0..3 head-groups of 4
heads). Each core: QKV projection for its 4 heads, banded attention, and
a partial output projection (its 256 columns of the head concat). Host
sums the partials, removes the dither correction, and rescales.

Numerics (all fp8 uses are residual-corrected except the single q
requantization):
  - Weights are scaled x16 before e4m3 quantization (their natural scale
    sits in e4m3's subnormal range); activations/V/outputs carry the x16
    factor through and the host divides by 256 at the end.
  - QKV projections: fp8 DoubleRow (contraction 2x128/matmul, 0.5
    cyc/col), 3 terms: x8@w8 + xe8@w8 + x8@we8 (x and w residuals).
  - Scores: fp8 DoubleRow per head; the two z-slots compute
    (k8 + ke8).q8, i.e. the k requantization is corrected in-slot for
    free. ke8 is produced by one extra DVE op per k chain. The q
    requantization is the one uncorrected noise source.
  - AV in fp16 (V carries 16x), both heads of a pair in one PSUM bank
    with per-region start/stop; one reciprocal + one stride-0 broadcast
    normalize per pair.
  - Out-projection in fp16 (o stays 16x; host divides by 16): fp8
    here is too noisy for the 2e-2 gate even dithered.
"""

import numpy as np
import ml_dtypes

import concourse.bacc as bacc
import concourse.mybir as mybir
import concourse.tile as tile
from concourse.bass_utils import run_bass_kernel_spmd
from concourse.masks import make_identity

F32 = mybir.dt.float32
F16 = mybir.dt.float16
F8 = mybir.dt.float8e4
E4M3 = ml_dtypes.float8_e4m3
DR = mybir.MatmulPerfMode.DoubleRow

P = 128
L = 2048
D = 1024
NH = 4          # heads per core
HD = 64
FV = 256        # v feature rows per core
WIN = 256
KB = L // P     # 16 k/token blocks
N_CORES = 8
WS = 16.0       # weight/activation scale before fp8
EXP_SCALE = 0.125 / (WS * WS)

C1_LAG = 3
D_LAG = 4


def _window(kb):
    k0 = kb * P
    qlo = max(0, k0 - WIN)
    qhi = min(L, k0 + P + WIN)
    return qlo, qhi - qlo


def _build_nc():
    nc = bacc.Bacc(
        "TRN2", target_bir_lowering=False, debug=False, num_devices=N_CORES
    )
    x8_d = nc.dram_tensor("x8", [P, 8, L], F8, kind="ExternalInput").ap()
    xe8_d = nc.dram_tensor("xe8", [P, 8, L], F8, kind="ExternalInput").ap()
    wqk_d = nc.dram_tensor("wqk8", [P, 4, 8, P], F8, kind="ExternalInput").ap()
    wqke_d = nc.dram_tensor("wqke8", [P, 4, 8, P], F8, kind="ExternalInput").ap()
    wv_d = nc.dram_tensor("wv8", [P, 8, FV], F8, kind="ExternalInput").ap()
    wve_d = nc.dram_tensor("wve8", [P, 8, FV], F8, kind="ExternalInput").ap()
    wo_d = nc.dram_tensor("wo16", [P, 2, D], F16, kind="ExternalInput").ap()
    bqk_d = nc.dram_tensor("bqk", [P, 4], F32, kind="ExternalInput").ap()
    bv_d = nc.dram_tensor("bv", [1, FV], F32, kind="ExternalInput").ap()
    y_d = nc.dram_tensor("y", [L, D], F16, kind="ExternalOutput").ap()

    with tile.TileContext(nc) as tc:
        _emit(nc, tc, x8_d, xe8_d, wqk_d, wqke_d, wv_d, wve_d, wo_d,
              bqk_d, bv_d, y_d)
    nc.compile()
    return nc


def _emit(nc, tc, x8_d, xe8_d, wqk_d, wqke_d, wv_d, wve_d, wo_d,
          bqk_d, bv_d, y_d):
    import contextlib

    ctx = contextlib.ExitStack()
    with ctx:
        const = ctx.enter_context(tc.tile_pool(name="const", bufs=1))
        w_pool = ctx.enter_context(tc.tile_pool(name="w", bufs=1))
        qk_pool = ctx.enter_context(tc.tile_pool(name="qk", bufs=1))
        v_pool = ctx.enter_context(tc.tile_pool(name="v", bufs=1))
        x_pool = ctx.enter_context(tc.tile_pool(name="x", bufs=1))
        e_pool = ctx.enter_context(tc.tile_pool(name="e", bufs=7))
        oT_pool = ctx.enter_context(tc.tile_pool(name="oT", bufs=1))
        opr_pool = ctx.enter_context(tc.tile_pool(name="opr", bufs=3))
        rr_pool = ctx.enter_context(tc.tile_pool(name="rr", bufs=4))
        ysb_pool = ctx.enter_context(tc.tile_pool(name="ysb", bufs=3))
        spool = ctx.enter_context(tc.tile_pool(name="spsum", bufs=2, space="PSUM"))
        pqpool = ctx.enter_context(tc.tile_pool(name="pqpsum", bufs=2, space="PSUM"))
        otpool = ctx.enter_context(tc.tile_pool(name="otpsum", bufs=2, space="PSUM"))

        # ---- input DMAs (pipeline-ordered) ------------------------------
        bqk_sb = const.tile([P, 4], F32)
        nc.sync.dma_start(bqk_sb[:], bqk_d[:])
        bv_row = const.tile([1, FV], F32)
        nc.sync.dma_start(bv_row[:], bv_d[:])

        wqk_sb = w_pool.tile([P, 4, 8, P], F8)
        wqke_sb = w_pool.tile([P, 4, 8, P], F8)
        x8_sb = x_pool.tile([P, 8, L], F8)
        xe8_sb = x_pool.tile([P, 8, L], F8)
        # startup: q/k weights for wave 0 first, then first x chunks
        nc.sync.dma_start(wqk_sb[:, 0, :, :], wqk_d[:, 0, :, :])
        nc.sync.dma_start(wqk_sb[:, 2, :, :], wqk_d[:, 2, :, :])
        for sl in (slice(0, 256), slice(256, 512)):
            nc.sync.dma_start(x8_sb[:, :, sl], x8_d[:, :, sl])
            nc.sync.dma_start(xe8_sb[:, :, sl], xe8_d[:, :, sl])
        nc.sync.dma_start(wqke_sb[:, 0, :, :], wqke_d[:, 0, :, :])
        nc.sync.dma_start(wqke_sb[:, 2, :, :], wqke_d[:, 2, :, :])
        nc.sync.dma_start(wqk_sb[:, 1, :, :], wqk_d[:, 1, :, :])
        nc.sync.dma_start(wqk_sb[:, 3, :, :], wqk_d[:, 3, :, :])
        nc.sync.dma_start(wqke_sb[:, 1, :, :], wqke_d[:, 1, :, :])
        nc.sync.dma_start(wqke_sb[:, 3, :, :], wqke_d[:, 3, :, :])
        for t in range(1, 4):
            sl = slice(t * 512, (t + 1) * 512)
            nc.sync.dma_start(x8_sb[:, :, sl], x8_d[:, :, sl])
            nc.sync.dma_start(xe8_sb[:, :, sl], xe8_d[:, :, sl])
            if t == 1:
                wv_sb = w_pool.tile([P, 8, FV], F8)
                nc.sync.dma_start(wv_sb[:], wv_d[:])
                wve_sb = w_pool.tile([P, 8, FV], F8)
                nc.sync.dma_start(wve_sb[:], wve_d[:])
                wo_sb = w_pool.tile([P, 2, D], F16)
                nc.sync.dma_start(wo_sb[:], wo_d[:])

        # ---- constants --------------------------------------------------
        ident_f32 = const.tile([P, P], F32)
        make_identity(nc, ident_f32[:])
        ident = const.tile([P, P], F16)
        nc.vector.tensor_copy(ident[:], ident_f32[:])
        bv_bc = const.tile([P, FV], F32)
        nc.gpsimd.partition_broadcast(bv_bc[:], bv_row[:])

        # Q^T (fp8, 16x): [dims(2 heads x 64), cc, token]
        qT8 = qk_pool.tile([P, 2, L], F8)
        # K^T with z-dim: z0 = k8, z1 = requant residual
        kT8 = qk_pool.tile([P, 2, 2, L], F8)
        v_ext = v_pool.tile([P, KB, NH * (HD + 1)], F16)
        nc.vector.memset(
            v_ext[:].rearrange("p b (h c) -> p b h c", h=NH)[:, :, :, HD:],
            1.0)
        oT = oT_pool.tile([P, 2, L], F16)

        # ---- emission helpers -------------------------------------------
        def qk_chain(fc, t, half=None):
            """12 fp8-DR matmuls (x8@w8, xe8@w8, x8@we8), fp8 bias-add;
            for k chains also the requant residual (z1)."""
            if half is None:
                tsl = slice(t * 512, (t + 1) * 512)
            else:
                tsl = slice(t * 512 + half * 256, t * 512 + half * 256 + 256)
            n = tsl.stop - tsl.start
            pq = pqpool.tile([P, 512], F32, tag="pq", name="pq")
            terms = ((x8_sb, wqk_sb), (xe8_sb, wqk_sb), (x8_sb, wqke_sb))
            for i, (xs, ws) in enumerate(terms):
                for d4 in range(4):
                    nc.tensor.matmul(
                        pq[:, 0:n],
                        lhsT=ws[:, fc, 2 * d4:2 * d4 + 2, :],
                        rhs=xs[:, 2 * d4:2 * d4 + 2, tsl],
                        start=(i == 0 and d4 == 0), stop=(i == 2 and d4 == 3),
                        perf_mode=DR,
                    )
            cc = fc % 2
            if fc < 2:
                nc.vector.tensor_scalar_add(
                    qT8[:, cc, tsl], pq[:, 0:n], bqk_sb[:, fc:fc + 1])
            else:
                nc.vector.tensor_scalar_add(
                    kT8[:, cc, 0, tsl], pq[:, 0:n], bqk_sb[:, fc:fc + 1])
                # z1 = (pq + bias) - k8   (requant residual)
                nc.vector.scalar_tensor_tensor(
                    kT8[:, cc, 1, tsl], pq[:, 0:n], bqk_sb[:, fc:fc + 1],
                    kT8[:, cc, 0, tsl],
                    mybir.AluOpType.add, mybir.AluOpType.subtract)

        def v_chain(tb):
            tsl = slice(tb * P, (tb + 1) * P)
            pv = pqpool.tile([P, 512], F32, tag="pq", name="pv")
            terms = ((x8_sb, wv_sb), (xe8_sb, wv_sb), (x8_sb, wve_sb))
            for i, (xs, ws) in enumerate(terms):
                for d4 in range(4):
                    nc.tensor.matmul(
                        pv[:, 0:FV],
                        lhsT=xs[:, 2 * d4:2 * d4 + 2, tsl],
                        rhs=ws[:, 2 * d4:2 * d4 + 2, :],
                        start=(i == 0 and d4 == 0), stop=(i == 2 and d4 == 3),
                        perf_mode=DR,
                    )
            nc.vector.tensor_add(
                v_ext[:, tb, :].rearrange("p (h c) -> p h c", h=NH)[:, :, 0:HD],
                pv[:, 0:FV].rearrange("p (h c) -> p h c", h=NH),
                bv_bc[:].rearrange("p (h c) -> p h c", h=NH),
            )

        def phase_b(h, kb, e_tiles):
            """Scores for one head: fp8-DR, z = (k8, k-residual)."""
            qlo, w = _window(kb)
            if h == 0:
                e4 = e_pool.tile([P, NH, 640], F16, tag="e", name="e4")
                e_tiles[kb] = e4
            else:
                e4 = e_tiles[kb]
            s_ps = spool.tile([P, 1024], F32, tag="s", name="s_ps")
            cc, hh = h // 2, h % 2
            base = 64 * hh
            if w == 640:
                pieces = [(0, 0, 320), (320, 512, 320)]
            else:
                pieces = [(0, 0, w)]
            lhsT = kT8[base:base + 64, cc, :, kb * P:(kb + 1) * P]
            for qoff, poff, pw in pieces:
                rhs = (qT8[base:base + 64, cc, qlo + qoff:qlo + qoff + pw]
                       .unsqueeze(1).broadcast_to([64, 2, pw]))
                nc.tensor.matmul(
                    s_ps[:, poff:poff + pw],
                    lhsT=lhsT, rhs=rhs, start=True, stop=True,
                    perf_mode=DR, skip_group_check=True,
                )
            if w == 640:
                src = s_ps[:].rearrange("p (g c) -> p g c", g=2)[:, :, 0:320]
                dst = e4[:, h, :].rearrange("p (g c) -> p g c", g=2)
            else:
                src = s_ps[:, 0:w]
                dst = e4[:, h, 0:w]
            nc.scalar.activation(
                dst, src, mybir.ActivationFunctionType.Exp, scale=EXP_SCALE)

        def masks(kb, e_tiles):
            e4 = e_tiles[kb]
            qlo, w = _window(kb)
            if kb >= 2:
                nc.gpsimd.affine_select(
                    out=e4[:, :, 0:P], in_=e4[:, :, 0:P],
                    compare_op=mybir.AluOpType.is_ge, fill=0.0,
                    base=0, pattern=[[0, NH], [1, P]], channel_multiplier=-1,
                )
            if kb <= KB - 3:
                nc.gpsimd.affine_select(
                    out=e4[:, :, w - P:w], in_=e4[:, :, w - P:w],
                    compare_op=mybir.AluOpType.is_ge, fill=0.0,
                    base=0, pattern=[[0, NH], [-1, P]], channel_multiplier=1,
                )

        def phase_c(pair, qt, e_tiles):
            """AV (pair in one bank), recip + stride-0 normalize, fp16
            transpose, dithered fp8 oT write."""
            kbs = range(max(0, qt - 2), min(KB, qt + 3))
            ot = otpool.tile([P, 512], F32, tag="ot", name="ot")
            for hh in range(2):
                h = 2 * pair + hh
                for i, kb in enumerate(kbs):
                    qlo, _ = _window(kb)
                    off = qt * P - qlo
                    nc.tensor.matmul(
                        ot[:, hh * 65:hh * 65 + 65],
                        lhsT=e_tiles[kb][:, h, off:off + P],
                        rhs=v_ext[:, kb, h * 65:h * 65 + 65],
                        start=(i == 0), stop=(i == len(kbs) - 1),
                        skip_group_check=True,
                    )
            o_pair = ot[:, 0:130].rearrange("p (h c) -> p h c", h=2)
            rr = rr_pool.tile([P, 2], F32, tag="rr", name="rr")
            nc.vector.reciprocal(rr[:], o_pair[:, :, HD:HD + 1])
            opr = opr_pool.tile([P, P], F16, tag="opr", name="opr")
            nc.vector.tensor_mul(
                opr[:].rearrange("p (h c) -> p h c", h=2),
                o_pair[:, :, 0:HD],
                rr[:].rearrange("p (h o) -> p h o", h=2).broadcast_to([P, 2, HD]),
            )
            t_ps = ot[:, 256:320].bitcast(F16)
            nc.tensor.transpose(t_ps, opr[:], ident[:])
            nc.vector.tensor_copy(oT[:, pair, qt * P:(qt + 1) * P], t_ps)

        def phase_d(qt):
            y_sb = ysb_pool.tile([P, D], F16, tag="ysb", name="y_sb")
            for half in range(2):
                y_ps = pqpool.tile([P, 512], F32, tag="pq", name="y_ps")
                hsl = slice(half * 512, (half + 1) * 512)
                for cc in range(2):
                    nc.tensor.matmul(
                        y_ps[:],
                        lhsT=oT[:, cc, qt * P:(qt + 1) * P],
                        rhs=wo_sb[:, cc, hsl],
                        start=(cc == 0), stop=(cc == 1),
                    )
                if half == 0:
                    nc.vector.tensor_copy(y_sb[:, hsl], y_ps[:])
                else:
                    nc.scalar.copy(y_sb[:, hsl], y_ps[:])
            nc.sync.dma_start(y_d[qt * P:(qt + 1) * P, :], y_sb[:])

        # ---- schedule ---------------------------------------------------
        proj_a, proj_b = {}, {}
        proj_a[0] = lambda: qk_chain(0, 1)
        proj_b[0] = lambda: qk_chain(2, 1)
        proj_a[1] = lambda: qk_chain(1, 1)
        proj_b[1] = lambda: qk_chain(3, 1)
        for i, (fc, t) in enumerate([(0, 2), (2, 2), (1, 2), (3, 2)]):
            proj_a[2 + i] = lambda fc=fc, t=t: qk_chain(fc, t)
        for i, (fc, t) in enumerate([(0, 3), (2, 3), (1, 3), (3, 3)]):
            proj_a[6 + i] = lambda fc=fc, t=t: qk_chain(fc, t)

        # prologue: wave 0 at 256-col granularity + v blocks 0-1
        for fc in (0, 2):
            for half in (0, 1):
                qk_chain(fc, 0, half=half)
        for fc in (1, 3):
            for half in (0, 1):
                qk_chain(fc, 0, half=half)
        v_chain(0)
        v_chain(1)

        e_tiles = {}
        for step in range(KB + C1_LAG + 2):
            kb = step if step < KB else None
            qt = step - C1_LAG
            qt2 = step - D_LAG
            if kb is not None:
                phase_b(0, kb, e_tiles)
                phase_b(1, kb, e_tiles)
            if step in proj_a:
                proj_a[step]()
            if qt in range(KB):
                phase_c(0, qt, e_tiles)
            if kb is not None:
                phase_b(2, kb, e_tiles)
                phase_b(3, kb, e_tiles)
            if step in proj_b:
                proj_b[step]()
            if kb is not None and kb + 2 < KB:
                v_chain(kb + 2)
            if qt in range(KB):
                phase_c(1, qt, e_tiles)
            if qt2 in range(KB):
                phase_d(qt2)
            if kb is not None:
                masks(kb, e_tiles)


_NC_CACHE = None


def _get_nc():
    global _NC_CACHE
    if _NC_CACHE is None:
        _NC_CACHE = _build_nc()
    return _NC_CACHE


def _fp8_split(a):
    hi = a.astype(E4M3)
    lo = (a - hi.astype(np.float32)).astype(E4M3)
    return hi, lo


def kernel(x, qkv_w, qkv_b, out_w, out_b):
    x = np.asarray(x, dtype=np.float32)
    qkv_w = np.asarray(qkv_w, dtype=np.float32)
    qkv_b = np.asarray(qkv_b, dtype=np.float32)
    out_w = np.asarray(out_w, dtype=np.float32)
    out_b = np.asarray(out_b, dtype=np.float32)
    B = x.shape[0]
    assert x.shape == (B, L, D) and B * 4 == N_CORES

    nc = _get_nc()

    xs = []
    for b in range(B):
        xt = np.ascontiguousarray(x[b].T)            # [D, L]
        x8, xe8 = _fp8_split(xt)
        xs.append((
            np.ascontiguousarray(x8.reshape(8, P, L).transpose(1, 0, 2)),
            np.ascontiguousarray(xe8.reshape(8, P, L).transpose(1, 0, 2)),
        ))

    in_maps = []
    for core in range(N_CORES):
        b, g = divmod(core, 4)
        rq = slice(g * FV, (g + 1) * FV)
        rk = slice(D + g * FV, D + (g + 1) * FV)
        rv = slice(2 * D + g * FV, 2 * D + (g + 1) * FV)
        wqk_t = np.concatenate([qkv_w[rq], qkv_w[rk]], axis=0).T * WS
        w8, we8 = _fp8_split(wqk_t)

        def to_fc(a):
            return np.ascontiguousarray(
                a.reshape(D, 4, P).transpose(1, 0, 2)
                .reshape(4, 8, P, P).transpose(2, 0, 1, 3))

        wv_t = qkv_w[rv].T * WS                       # [D, 256]
        wv8, wve8 = _fp8_split(wv_t)

        def to_v(a):
            return np.ascontiguousarray(a.reshape(8, P, FV).transpose(1, 0, 2))

        wo16 = out_w[:, g * FV:(g + 1) * FV].T.astype(np.float16)  # [256, D]
        wo16 = np.ascontiguousarray(wo16.reshape(2, P, D).transpose(1, 0, 2))

        bqk = np.ascontiguousarray(
            (WS * np.concatenate([qkv_b[rq], qkv_b[rk]])).reshape(4, P).T)
        bv = np.ascontiguousarray((WS * qkv_b[rv]).reshape(1, FV))
        in_maps.append({
            "x8": xs[b][0], "xe8": xs[b][1],
            "wqk8": to_fc(w8), "wqke8": to_fc(we8),
            "wv8": to_v(wv8), "wve8": to_v(wve8),
            "wo16": wo16,
            "bqk": bqk, "bv": bv,
        })

    res = run_bass_kernel_spmd(nc, in_maps, list(range(N_CORES)))
    y = np.empty((B, L, D), dtype=np.float32)
    for b in range(B):
        acc = res.results[b * 4 + 0]["y"].astype(np.float32)
        for g in range(1, 4):
            acc = acc + res.results[b * 4 + g]["y"]
        y[b] = acc / WS
    if np.any(out_b):
        y += out_b
    return y


# revision 14
# speedup vs baseline: 1.1109x; 1.0087x over previous
"""Sliding-window MHA Trainium2 kernel, sharded over 8 NeuronCores.

Problem (hardcoded): B=2, L=2048, D=1024, H=16 heads (hd=64), window
|i-j| <= 256, fp32 I/O.

Sharding: core = b*4 + g (b in 0..1 batches, g in 0..3 head-groups of 4
heads). Each core: QKV projection for its 4 heads, banded attention, and
a partial output projection (its 256 columns of the head concat). Host
sums the partials, removes the dither correction, and rescales.

Numerics (all fp8 uses are residual-corrected except the single q
requantization):
  - Weights are scaled x16 before e4m3 quantization (their natural scale
    sits in e4m3's subnormal range); activations/V/outputs carry the x16
    factor through and the host divides by 256 at the end.
  - QKV projections: fp8 DoubleRow (contraction 2x128/matmul, 0.5
    cyc/col), 3 terms: x8@w8 + xe8@w8 + x8@we8 (x and w residuals).
  - Scores: fp8 DoubleRow per head; the two z-slots compute
    (k8 + ke8).q8, i.e. the k requantization is corrected in-slot for
    free. ke8 is produced by one extra DVE op per k chain. The q
    requantization is the one uncorrected noise source.
  - AV in fp16 (V carries 16x), both heads of a pair in one PSUM bank
    with per-region start/stop; one reciprocal + one stride-0 broadcast
    normalize per pair.
  - Out-projection in fp16 (o stays 16x; host divides by 16): fp8
    here is too noisy for the 2e-2 gate even dithered.
"""

import numpy as np
import ml_dtypes

import concourse.bacc as bacc
import concourse.mybir as mybir
import concourse.tile as tile
from concourse.bass_utils import run_bass_kernel_spmd
from concourse.masks import make_identity

F32 = mybir.dt.float32
F16 = mybir.dt.float16
F8 = mybir.dt.float8e4
E4M3 = ml_dtypes.float8_e4m3
DR = mybir.MatmulPerfMode.DoubleRow

P = 128
L = 2048
D = 1024
NH = 4          # heads per core
HD = 64
FV = 256        # v feature rows per core
WIN = 256
KB = L // P     # 16 k/token blocks
N_CORES = 8
WS = 16.0       # weight/activation scale before fp8
EXP_SCALE = 0.125 / (WS * WS)

C1_LAG = 3
D_LAG = 4


def _window(kb):
    k0 = kb * P
    qlo = max(0, k0 - WIN)
    qhi = min(L, k0 + P + WIN)
    return qlo, qhi - qlo


def _build_nc():
    nc = bacc.Bacc(
        "TRN2", target_bir_lowering=False, debug=False, num_devices=N_CORES
    )
    x8_d = nc.dram_tensor("x8", [P, 8, L], F8, kind="ExternalInput").ap()
    xe8_d = nc.dram_tensor("xe8", [P, 8, L], F8, kind="ExternalInput").ap()
    wqk_d = nc.dram_tensor("wqk8", [P, 4, 8, P], F8, kind="ExternalInput").ap()
    wqke_d = nc.dram_tensor("wqke8", [P, 4, 8, P], F8, kind="ExternalInput").ap()
    wv_d = nc.dram_tensor("wv8", [P, 8, FV], F8, kind="ExternalInput").ap()
    wve_d = nc.dram_tensor("wve8", [P, 8, FV], F8, kind="ExternalInput").ap()
    wo_d = nc.dram_tensor("wo16", [P, 2, D], F16, kind="ExternalInput").ap()
    bqk_d = nc.dram_tensor("bqk", [P, 4], F32, kind="ExternalInput").ap()
    bv_d = nc.dram_tensor("bv", [1, FV], F32, kind="ExternalInput").ap()
    y_d = nc.dram_tensor("y", [L, D], F16, kind="ExternalOutput").ap()

    with tile.TileContext(nc) as tc:
        _emit(nc, tc, x8_d, xe8_d, wqk_d, wqke_d, wv_d, wve_d, wo_d,
              bqk_d, bv_d, y_d)
    nc.compile()
    return nc


def _emit(nc, tc, x8_d, xe8_d, wqk_d, wqke_d, wv_d, wve_d, wo_d,
          bqk_d, bv_d, y_d):
    import contextlib

    ctx = contextlib.ExitStack()
    with ctx:
        const = ctx.enter_context(tc.tile_pool(name="const", bufs=1))
        w_pool = ctx.enter_context(tc.tile_pool(name="w", bufs=1))
        qk_pool = ctx.enter_context(tc.tile_pool(name="qk", bufs=1))
        v_pool = ctx.enter_context(tc.tile_pool(name="v", bufs=1))
        x_pool = ctx.enter_context(tc.tile_pool(name="x", bufs=1))
        e_pool = ctx.enter_context(tc.tile_pool(name="e", bufs=7))
        oT_pool = ctx.enter_context(tc.tile_pool(name="oT", bufs=1))
        opr_pool = ctx.enter_context(tc.tile_pool(name="opr", bufs=3))
        rr_pool = ctx.enter_context(tc.tile_pool(name="rr", bufs=4))
        ysb_pool = ctx.enter_context(tc.tile_pool(name="ysb", bufs=3))
        spool = ctx.enter_context(tc.tile_pool(name="spsum", bufs=2, space="PSUM"))
        pqpool = ctx.enter_context(tc.tile_pool(name="pqpsum", bufs=2, space="PSUM"))
        otpool = ctx.enter_context(tc.tile_pool(name="otpsum", bufs=2, space="PSUM"))

        # ---- input DMAs (pipeline-ordered) ------------------------------
        bqk_sb = const.tile([P, 4], F32)
        nc.sync.dma_start(bqk_sb[:], bqk_d[:])
        bv_row = const.tile([1, FV], F32)
        nc.sync.dma_start(bv_row[:], bv_d[:])

        wqk_sb = w_pool.tile([P, 4, 8, P], F8)
        wqke_sb = w_pool.tile([P, 4, 8, P], F8)
        x8_sb = x_pool.tile([P, 8, L], F8)
        xe8_sb = x_pool.tile([P, 8, L], F8)
        # startup order: first x chunk, then weights fc by fc, then the
        # rest of x interleaved with v/o weights
        sl0 = slice(0, 512)
        nc.sync.dma_start(x8_sb[:, :, sl0], x8_d[:, :, sl0])
        nc.sync.dma_start(wqk_sb[:, 0, :, :], wqk_d[:, 0, :, :])
        nc.sync.dma_start(xe8_sb[:, :, sl0], xe8_d[:, :, sl0])
        nc.sync.dma_start(wqke_sb[:, 0, :, :], wqke_d[:, 0, :, :])
        wv_sb = w_pool.tile([P, 8, FV], F8)
        wve_sb = w_pool.tile([P, 8, FV], F8)
        wo_sb = w_pool.tile([P, 2, D], F16)
        for fc in (2, 1, 3):
            nc.sync.dma_start(wqk_sb[:, fc, :, :], wqk_d[:, fc, :, :])
            nc.sync.dma_start(wqke_sb[:, fc, :, :], wqke_d[:, fc, :, :])
            if fc == 1:
                nc.sync.dma_start(wv_sb[:], wv_d[:])
                nc.sync.dma_start(wve_sb[:], wve_d[:])
        for t in range(1, 4):
            sl = slice(t * 512, (t + 1) * 512)
            nc.sync.dma_start(x8_sb[:, :, sl], x8_d[:, :, sl])
            nc.sync.dma_start(xe8_sb[:, :, sl], xe8_d[:, :, sl])
            if t == 2:
                nc.sync.dma_start(wo_sb[:], wo_d[:])

        # ---- constants --------------------------------------------------
        ident_f32 = const.tile([P, P], F32)
        make_identity(nc, ident_f32[:])
        ident = const.tile([P, P], F16)
        nc.vector.tensor_copy(ident[:], ident_f32[:])
        bv_bc = const.tile([P, FV], F32)
        nc.gpsimd.partition_broadcast(bv_bc[:], bv_row[:])

        # Q^T (fp8, 16x): [dims(2 heads x 64), cc, token]
        qT8 = qk_pool.tile([P, 2, L], F8)
        # K^T with z-dim: z0 = k8, z1 = requant residual
        kT8 = qk_pool.tile([P, 2, 2, L], F8)
        v_ext = v_pool.tile([P, KB, NH * (HD + 1)], F16)
        nc.vector.memset(
            v_ext[:].rearrange("p b (h c) -> p b h c", h=NH)[:, :, :, HD:],
            1.0)
        oT = oT_pool.tile([P, 2, L], F16)

        # ---- emission helpers -------------------------------------------
        def qk_chain(fc, t, half=None):
            """12 fp8-DR matmuls (x8@w8, xe8@w8, x8@we8), fp8 bias-add;
            for k chains also the requant residual (z1)."""
            if half is None:
                tsl = slice(t * 512, (t + 1) * 512)
            else:
                tsl = slice(t * 512 + half * 256, t * 512 + half * 256 + 256)
            n = tsl.stop - tsl.start
            pq = pqpool.tile([P, 512], F32, tag="pq", name="pq")
            terms = ((x8_sb, wqk_sb), (xe8_sb, wqk_sb), (x8_sb, wqke_sb))
            for i, (xs, ws) in enumerate(terms):
                for d4 in range(4):
                    nc.tensor.matmul(
                        pq[:, 0:n],
                        lhsT=ws[:, fc, 2 * d4:2 * d4 + 2, :],
                        rhs=xs[:, 2 * d4:2 * d4 + 2, tsl],
                        start=(i == 0 and d4 == 0), stop=(i == 2 and d4 == 3),
                        perf_mode=DR,
                    )
            cc = fc % 2
            if fc < 2:
                nc.vector.tensor_scalar_add(
                    qT8[:, cc, tsl], pq[:, 0:n], bqk_sb[:, fc:fc + 1])
            else:
                nc.vector.tensor_scalar_add(
                    kT8[:, cc, 0, tsl], pq[:, 0:n], bqk_sb[:, fc:fc + 1])
                # z1 = (pq + bias) - k8   (requant residual)
                nc.vector.scalar_tensor_tensor(
                    kT8[:, cc, 1, tsl], pq[:, 0:n], bqk_sb[:, fc:fc + 1],
                    kT8[:, cc, 0, tsl],
                    mybir.AluOpType.add, mybir.AluOpType.subtract)

        def v_chain(tb):
            tsl = slice(tb * P, (tb + 1) * P)
            pv = pqpool.tile([P, 512], F32, tag="pq", name="pv")
            terms = ((x8_sb, wv_sb), (xe8_sb, wv_sb), (x8_sb, wve_sb))
            for i, (xs, ws) in enumerate(terms):
                for d4 in range(4):
                    nc.tensor.matmul(
                        pv[:, 0:FV],
                        lhsT=xs[:, 2 * d4:2 * d4 + 2, tsl],
                        rhs=ws[:, 2 * d4:2 * d4 + 2, :],
                        start=(i == 0 and d4 == 0), stop=(i == 2 and d4 == 3),
                        perf_mode=DR,
                    )
            nc.vector.tensor_add(
                v_ext[:, tb, :].rearrange("p (h c) -> p h c", h=NH)[:, :, 0:HD],
                pv[:, 0:FV].rearrange("p (h c) -> p h c", h=NH),
                bv_bc[:].rearrange("p (h c) -> p h c", h=NH),
            )

        def phase_b(h, kb, e_tiles):
            """Scores for one head: fp8-DR, z = (k8, k-residual)."""
            qlo, w = _window(kb)
            if h == 0:
                e4 = e_pool.tile([P, NH, 640], F16, tag="e", name="e4")
                e_tiles[kb] = e4
            else:
                e4 = e_tiles[kb]
            s_ps = spool.tile([P, 1024], F32, tag="s", name="s_ps")
            cc, hh = h // 2, h % 2
            base = 64 * hh
            if w == 640:
                pieces = [(0, 0, 320), (320, 512, 320)]
            else:
                pieces = [(0, 0, w)]
            lhsT = kT8[base:base + 64, cc, :, kb * P:(kb + 1) * P]
            for qoff, poff, pw in pieces:
                rhs = (qT8[base:base + 64, cc, qlo + qoff:qlo + qoff + pw]
                       .unsqueeze(1).broadcast_to([64, 2, pw]))
                nc.tensor.matmul(
                    s_ps[:, poff:poff + pw],
                    lhsT=lhsT, rhs=rhs, start=True, stop=True,
                    perf_mode=DR, skip_group_check=True,
                )
            if w == 640:
                src = s_ps[:].rearrange("p (g c) -> p g c", g=2)[:, :, 0:320]
                dst = e4[:, h, :].rearrange("p (g c) -> p g c", g=2)
            else:
                src = s_ps[:, 0:w]
                dst = e4[:, h, 0:w]
            nc.scalar.activation(
                dst, src, mybir.ActivationFunctionType.Exp, scale=EXP_SCALE)

        def masks(kb, e_tiles):
            e4 = e_tiles[kb]
            qlo, w = _window(kb)
            if kb >= 2:
                nc.gpsimd.affine_select(
                    out=e4[:, :, 0:P], in_=e4[:, :, 0:P],
                    compare_op=mybir.AluOpType.is_ge, fill=0.0,
                    base=0, pattern=[[0, NH], [1, P]], channel_multiplier=-1,
                )
            if kb <= KB - 3:
                nc.gpsimd.affine_select(
                    out=e4[:, :, w - P:w], in_=e4[:, :, w - P:w],
                    compare_op=mybir.AluOpType.is_ge, fill=0.0,
                    base=0, pattern=[[0, NH], [-1, P]], channel_multiplier=1,
                )

        def phase_c(pair, qt, e_tiles):
            """AV (pair in one bank), recip + stride-0 normalize, fp16
            transpose, dithered fp8 oT write."""
            kbs = range(max(0, qt - 2), min(KB, qt + 3))
            ot = otpool.tile([P, 512], F32, tag="ot", name="ot")
            for hh in range(2):
                h = 2 * pair + hh
                for i, kb in enumerate(kbs):
                    qlo, _ = _window(kb)
                    off = qt * P - qlo
                    nc.tensor.matmul(
                        ot[:, hh * 65:hh * 65 + 65],
                        lhsT=e_tiles[kb][:, h, off:off + P],
                        rhs=v_ext[:, kb, h * 65:h * 65 + 65],
                        start=(i == 0), stop=(i == len(kbs) - 1),
                        skip_group_check=True,
                    )
            o_pair = ot[:, 0:130].rearrange("p (h c) -> p h c", h=2)
            rr = rr_pool.tile([P, 2], F32, tag="rr", name="rr")
            nc.vector.reciprocal(rr[:], o_pair[:, :, HD:HD + 1])
            opr = opr_pool.tile([P, P], F16, tag="opr", name="opr")
            nc.vector.tensor_mul(
                opr[:].rearrange("p (h c) -> p h c", h=2),
                o_pair[:, :, 0:HD],
                rr[:].rearrange("p (h o) -> p h o", h=2).broadcast_to([P, 2, HD]),
            )
            t_ps = ot[:, 256:320].bitcast(F16)
            nc.tensor.transpose(t_ps, opr[:], ident[:])
            nc.vector.tensor_copy(oT[:, pair, qt * P:(qt + 1) * P], t_ps)

        def phase_d(qt):
            y_sb = ysb_pool.tile([P, D], F16, tag="ysb", name="y_sb")
            tail = qt >= KB - 2
            for half in range(2):
                y_ps = pqpool.tile([P, 512], F32, tag="pq", name="y_ps")
                hsl = slice(half * 512, (half + 1) * 512)
                for cc in range(2):
                    nc.tensor.matmul(
                        y_ps[:],
                        lhsT=oT[:, cc, qt * P:(qt + 1) * P],
                        rhs=wo_sb[:, cc, hsl],
                        start=(cc == 0), stop=(cc == 1),
                    )
                if tail:
                    # quarter copies on alternating engines + half DMAs
                    q0 = slice(half * 512, half * 512 + 256)
                    q1 = slice(half * 512 + 256, half * 512 + 512)
                    nc.vector.tensor_copy(y_sb[:, q0], y_ps[:, 0:256])
                    nc.scalar.copy(y_sb[:, q1], y_ps[:, 256:512])
                    nc.sync.dma_start(y_d[qt * P:(qt + 1) * P, hsl],
                                      y_sb[:, hsl])
                elif half == 0:
                    nc.vector.tensor_copy(y_sb[:, hsl], y_ps[:])
                else:
                    nc.scalar.copy(y_sb[:, hsl], y_ps[:])
            if not tail:
                nc.sync.dma_start(y_d[qt * P:(qt + 1) * P, :], y_sb[:])

        # ---- schedule ---------------------------------------------------
        proj_a, proj_b = {}, {}
        proj_a[1] = [lambda: qk_chain(0, 1), lambda: qk_chain(2, 1)]
        proj_b[1] = [lambda: qk_chain(1, 1), lambda: qk_chain(3, 1)]
        proj_a[3] = [lambda: qk_chain(0, 2)]
        proj_b[3] = [lambda: qk_chain(2, 2)]
        proj_a[4] = [lambda: qk_chain(1, 2)]
        proj_b[4] = [lambda: qk_chain(3, 2)]
        proj_a[6] = [lambda: qk_chain(0, 3)]
        proj_b[6] = [lambda: qk_chain(2, 3)]
        proj_a[7] = [lambda: qk_chain(1, 3)]
        proj_b[7] = [lambda: qk_chain(3, 3)]
        # v blocks: emitted in the early slot (before phase_c of the
        # step), respecting wv DMA arrival (~step 2)
        v_sched_a = {3: [2], 4: [3, 4]}
        for s in range(5, KB):
            v_sched_a[s] = [s]
        v_sched_b = {2: [0, 1]}

        # prologue: wave 0 chains
        for fc in (0, 2, 1, 3):
            qk_chain(fc, 0)

        e_tiles = {}
        for step in range(KB + C1_LAG + 2):
            kb = step if step < KB else None
            qt = step - C1_LAG
            qt2 = step - D_LAG
            if kb is not None:
                phase_b(0, kb, e_tiles)
                phase_b(1, kb, e_tiles)
            for f in proj_a.get(step, ()):
                f()
            for n in v_sched_a.get(step, ()):
                v_chain(n)
            if qt in range(KB):
                phase_c(0, qt, e_tiles)
            if kb is not None:
                phase_b(2, kb, e_tiles)
                phase_b(3, kb, e_tiles)
            for f in proj_b.get(step, ()):
                f()
            for n in v_sched_b.get(step, ()):
                v_chain(n)
            if qt in range(KB):
                phase_c(1, qt, e_tiles)
            if qt2 in range(KB):
                phase_d(qt2)
            if kb is not None:
                masks(kb, e_tiles)


_NC_CACHE = None


def _get_nc():
    global _NC_CACHE
    if _NC_CACHE is None:
        _NC_CACHE = _build_nc()
    return _NC_CACHE


def _fp8_split(a):
    hi = a.astype(E4M3)
    lo = (a - hi.astype(np.float32)).astype(E4M3)
    return hi, lo


def kernel(x, qkv_w, qkv_b, out_w, out_b):
    x = np.asarray(x, dtype=np.float32)
    qkv_w = np.asarray(qkv_w, dtype=np.float32)
    qkv_b = np.asarray(qkv_b, dtype=np.float32)
    out_w = np.asarray(out_w, dtype=np.float32)
    out_b = np.asarray(out_b, dtype=np.float32)
    B = x.shape[0]
    assert x.shape == (B, L, D) and B * 4 == N_CORES

    nc = _get_nc()

    xs = []
    for b in range(B):
        xt = np.ascontiguousarray(x[b].T)            # [D, L]
        x8, xe8 = _fp8_split(xt)
        xs.append((
            np.ascontiguousarray(x8.reshape(8, P, L).transpose(1, 0, 2)),
            np.ascontiguousarray(xe8.reshape(8, P, L).transpose(1, 0, 2)),
        ))

    in_maps = []
    for core in range(N_CORES):
        b, g = divmod(core, 4)
        rq = slice(g * FV, (g + 1) * FV)
        rk = slice(D + g * FV, D + (g + 1) * FV)
        rv = slice(2 * D + g * FV, 2 * D + (g + 1) * FV)
        wqk_t = np.concatenate([qkv_w[rq], qkv_w[rk]], axis=0).T * WS
        w8, we8 = _fp8_split(wqk_t)

        def to_fc(a):
            return np.ascontiguousarray(
                a.reshape(D, 4, P).transpose(1, 0, 2)
                .reshape(4, 8, P, P).transpose(2, 0, 1, 3))

        wv_t = qkv_w[rv].T * WS                       # [D, 256]
        wv8, wve8 = _fp8_split(wv_t)

        def to_v(a):
            return np.ascontiguousarray(a.reshape(8, P, FV).transpose(1, 0, 2))

        wo16 = out_w[:, g * FV:(g + 1) * FV].T.astype(np.float16)  # [256, D]
        wo16 = np.ascontiguousarray(wo16.reshape(2, P, D).transpose(1, 0, 2))

        bqk = np.ascontiguousarray(
            (WS * np.concatenate([qkv_b[rq], qkv_b[rk]])).reshape(4, P).T)
        bv = np.ascontiguousarray((WS * qkv_b[rv]).reshape(1, FV))
        in_maps.append({
            "x8": xs[b][0], "xe8": xs[b][1],
            "wqk8": to_fc(w8), "wqke8": to_fc(we8),
            "wv8": to_v(wv8), "wve8": to_v(wve8),
            "wo16": wo16,
            "bqk": bqk, "bv": bv,
        })

    res = run_bass_kernel_spmd(nc, in_maps, list(range(N_CORES)))
    y = np.empty((B, L, D), dtype=np.float32)
    for b in range(B):
        acc = res.results[b * 4 + 0]["y"].astype(np.float32)
        for g in range(1, 4):
            acc = acc + res.results[b * 4 + g]["y"]
        y[b] = acc / WS
    if np.any(out_b):
        y += out_b
    return y


# revision 15
# speedup vs baseline: 1.1862x; 1.0678x over previous
"""Sliding-window MHA Trainium2 kernel, sharded over 8 NeuronCores.

Problem (hardcoded): B=2, L=2048, D=1024, H=16 heads (hd=64), window
|i-j| <= 256, fp32 I/O.

Sharding: core = b*4 + g (b in 0..1 batches, g in 0..3 head-groups of 4
heads). Each core: QKV projection for its 4 heads, banded attention, and
a partial output projection (its 256 columns of the head concat). Host
sums the partials, removes the dither correction, and rescales.

Numerics (all fp8 uses are residual-corrected except the single q
requantization):
  - Weights are scaled x16 before e4m3 quantization (their natural scale
    sits in e4m3's subnormal range); activations/V/outputs carry the x16
    factor through and the host divides by 256 at the end.
  - QKV projections: fp8 DoubleRow (contraction 2x128/matmul, 0.5
    cyc/col), 3 terms: x8@w8 + xe8@w8 + x8@we8 (x and w residuals).
  - Scores: fp8 DoubleRow per head; the two z-slots compute
    (k8 + ke8).q8, i.e. the k requantization is corrected in-slot for
    free. ke8 is produced by one extra DVE op per k chain. The q
    requantization is the one uncorrected noise source.
  - AV in fp16 (V carries 16x), both heads of a pair in one PSUM bank
    with per-region start/stop; one reciprocal + one stride-0 broadcast
    normalize per pair.
  - Out-projection in fp16 (o stays 16x; host divides by 16): fp8
    here is too noisy for the 2e-2 gate even dithered.
"""

import numpy as np
import ml_dtypes

import concourse.bacc as bacc
import concourse.mybir as mybir
import concourse.tile as tile
from concourse.bass_utils import run_bass_kernel_spmd
from concourse.masks import make_identity

F32 = mybir.dt.float32
F16 = mybir.dt.float16
F8 = mybir.dt.float8e4
E4M3 = ml_dtypes.float8_e4m3
DR = mybir.MatmulPerfMode.DoubleRow

P = 128
L = 2048
D = 1024
NH = 4          # heads per core
HD = 64
FV = 256        # v feature rows per core
WIN = 256
KB = L // P     # 16 k/token blocks
N_CORES = 8
WS = 16.0       # weight/activation scale before fp8
EXP_SCALE = 0.125 / (WS * WS)

C1_LAG = 3
D_LAG = 4


def _window(kb):
    k0 = kb * P
    qlo = max(0, k0 - WIN)
    qhi = min(L, k0 + P + WIN)
    return qlo, qhi - qlo


def _build_nc():
    nc = bacc.Bacc(
        "TRN2", target_bir_lowering=False, debug=False, num_devices=N_CORES
    )
    x8_d = nc.dram_tensor("x8", [P, 8, L], F8, kind="ExternalInput").ap()
    xe8_d = nc.dram_tensor("xe8", [P, 8, L], F8, kind="ExternalInput").ap()
    wqk_d = nc.dram_tensor("wqk8", [P, 4, 8, P], F8, kind="ExternalInput").ap()
    wqke_d = nc.dram_tensor("wqke8", [P, 4, 8, P], F8, kind="ExternalInput").ap()
    wv_d = nc.dram_tensor("wv8", [P, 8, FV], F8, kind="ExternalInput").ap()
    wve_d = nc.dram_tensor("wve8", [P, 8, FV], F8, kind="ExternalInput").ap()
    wo_d = nc.dram_tensor("wo16", [P, 2, D], F16, kind="ExternalInput").ap()
    bqk_d = nc.dram_tensor("bqk", [P, 4], F32, kind="ExternalInput").ap()
    bv_d = nc.dram_tensor("bv", [1, FV], F32, kind="ExternalInput").ap()
    y_d = nc.dram_tensor("y", [L, D], F16, kind="ExternalOutput").ap()

    with tile.TileContext(nc) as tc:
        _emit(nc, tc, x8_d, xe8_d, wqk_d, wqke_d, wv_d, wve_d, wo_d,
              bqk_d, bv_d, y_d)
    nc.compile()
    return nc


def _emit(nc, tc, x8_d, xe8_d, wqk_d, wqke_d, wv_d, wve_d, wo_d,
          bqk_d, bv_d, y_d):
    import contextlib

    ctx = contextlib.ExitStack()
    with ctx:
        const = ctx.enter_context(tc.tile_pool(name="const", bufs=1))
        w_pool = ctx.enter_context(tc.tile_pool(name="w", bufs=1))
        qk_pool = ctx.enter_context(tc.tile_pool(name="qk", bufs=1))
        v_pool = ctx.enter_context(tc.tile_pool(name="v", bufs=1))
        x_pool = ctx.enter_context(tc.tile_pool(name="x", bufs=1))
        e_pool = ctx.enter_context(tc.tile_pool(name="e", bufs=7))
        oT_pool = ctx.enter_context(tc.tile_pool(name="oT", bufs=1))
        opr_pool = ctx.enter_context(tc.tile_pool(name="opr", bufs=3))
        rr_pool = ctx.enter_context(tc.tile_pool(name="rr", bufs=4))
        ysb_pool = ctx.enter_context(tc.tile_pool(name="ysb", bufs=3))
        spool = ctx.enter_context(tc.tile_pool(name="spsum", bufs=2, space="PSUM"))
        pqpool = ctx.enter_context(tc.tile_pool(name="pqpsum", bufs=2, space="PSUM"))
        otpool = ctx.enter_context(tc.tile_pool(name="otpsum", bufs=2, space="PSUM"))

        # ---- input DMAs (pipeline-ordered) ------------------------------
        bqk_sb = const.tile([P, 4], F32)
        bv_row = const.tile([1, FV], F32)
        wqk_sb = w_pool.tile([P, 4, 8, P], F8)
        wqke_sb = w_pool.tile([P, 4, 8, P], F8)
        x8_sb = x_pool.tile([P, 8, L], F8)
        xe8_sb = x_pool.tile([P, 8, L], F8)
        # startup order: first x chunk, then weights fc by fc, then the
        # rest of x interleaved with v/o weights
        sl0 = slice(0, 512)
        nc.sync.dma_start(x8_sb[:, :, sl0], x8_d[:, :, sl0])
        nc.sync.dma_start(wqk_sb[:, 0, :, :], wqk_d[:, 0, :, :])
        nc.sync.dma_start(wqke_sb[:, 0, :, :], wqke_d[:, 0, :, :])
        nc.sync.dma_start(xe8_sb[:, :, sl0], xe8_d[:, :, sl0])
        nc.sync.dma_start(bqk_sb[:], bqk_d[:])
        nc.sync.dma_start(bv_row[:], bv_d[:])
        wv_sb = w_pool.tile([P, 8, FV], F8)
        wve_sb = w_pool.tile([P, 8, FV], F8)
        wo_sb = w_pool.tile([P, 2, D], F16)
        for fc in (2, 1, 3):
            nc.sync.dma_start(wqk_sb[:, fc, :, :], wqk_d[:, fc, :, :])
            nc.sync.dma_start(wqke_sb[:, fc, :, :], wqke_d[:, fc, :, :])
            if fc == 1:
                nc.sync.dma_start(wv_sb[:], wv_d[:])
                nc.sync.dma_start(wve_sb[:], wve_d[:])
        for t in range(1, 4):
            sl = slice(t * 512, (t + 1) * 512)
            nc.sync.dma_start(x8_sb[:, :, sl], x8_d[:, :, sl])
            nc.sync.dma_start(xe8_sb[:, :, sl], xe8_d[:, :, sl])
            if t == 2:
                nc.sync.dma_start(wo_sb[:], wo_d[:])

        # ---- constants --------------------------------------------------
        ident_f32 = const.tile([P, P], F32)
        make_identity(nc, ident_f32[:])
        ident = const.tile([P, P], F16)
        nc.vector.tensor_copy(ident[:], ident_f32[:])
        bv_bc = const.tile([P, FV], F32)
        nc.gpsimd.partition_broadcast(bv_bc[:], bv_row[:])

        # Q^T (fp8, 16x): [dims(2 heads x 64), cc, token]
        qT8 = qk_pool.tile([P, 2, L], F8)
        # K^T with z-dim: z0 = k8, z1 = requant residual
        kT8 = qk_pool.tile([P, 2, 2, L], F8)
        v_ext = v_pool.tile([P, KB, NH * (HD + 1)], F16)
        nc.vector.memset(
            v_ext[:].rearrange("p b (h c) -> p b h c", h=NH)[:, :, :, HD:],
            1.0)
        oT = oT_pool.tile([P, 2, L], F16)

        # ---- emission helpers -------------------------------------------
        def qk_chain(fc, t, half=None):
            """12 fp8-DR matmuls (x8@w8, xe8@w8, x8@we8), fp8 bias-add;
            for k chains also the requant residual (z1)."""
            if half is None:
                tsl = slice(t * 512, (t + 1) * 512)
            else:
                tsl = slice(t * 512 + half * 256, t * 512 + half * 256 + 256)
            n = tsl.stop - tsl.start
            pq = pqpool.tile([P, 512], F32, tag="pq", name="pq")
            terms = ((x8_sb, wqk_sb), (x8_sb, wqke_sb), (xe8_sb, wqk_sb))
            for i, (xs, ws) in enumerate(terms):
                for d4 in range(4):
                    nc.tensor.matmul(
                        pq[:, 0:n],
                        lhsT=ws[:, fc, 2 * d4:2 * d4 + 2, :],
                        rhs=xs[:, 2 * d4:2 * d4 + 2, tsl],
                        start=(i == 0 and d4 == 0), stop=(i == 2 and d4 == 3),
                        perf_mode=DR,
                    )
            cc = fc % 2
            if fc < 2:
                nc.vector.tensor_scalar_add(
                    qT8[:, cc, tsl], pq[:, 0:n], bqk_sb[:, fc:fc + 1])
            else:
                nc.vector.tensor_scalar_add(
                    kT8[:, cc, 0, tsl], pq[:, 0:n], bqk_sb[:, fc:fc + 1])
                # z1 = (pq + bias) - k8   (requant residual)
                nc.vector.scalar_tensor_tensor(
                    kT8[:, cc, 1, tsl], pq[:, 0:n], bqk_sb[:, fc:fc + 1],
                    kT8[:, cc, 0, tsl],
                    mybir.AluOpType.add, mybir.AluOpType.subtract)

        def v_chain(tb):
            tsl = slice(tb * P, (tb + 1) * P)
            pv = pqpool.tile([P, 512], F32, tag="pq", name="pv")
            terms = ((x8_sb, wv_sb), (x8_sb, wve_sb), (xe8_sb, wv_sb))
            for i, (xs, ws) in enumerate(terms):
                for d4 in range(4):
                    nc.tensor.matmul(
                        pv[:, 0:FV],
                        lhsT=xs[:, 2 * d4:2 * d4 + 2, tsl],
                        rhs=ws[:, 2 * d4:2 * d4 + 2, :],
                        start=(i == 0 and d4 == 0), stop=(i == 2 and d4 == 3),
                        perf_mode=DR,
                    )
            nc.vector.tensor_add(
                v_ext[:, tb, :].rearrange("p (h c) -> p h c", h=NH)[:, :, 0:HD],
                pv[:, 0:FV].rearrange("p (h c) -> p h c", h=NH),
                bv_bc[:].rearrange("p (h c) -> p h c", h=NH),
            )

        def phase_b(h, kb, e_tiles):
            """Scores for one head: fp8-DR, z = (k8, k-residual)."""
            qlo, w = _window(kb)
            if h == 0:
                e4 = e_pool.tile([P, NH, 640], F16, tag="e", name="e4")
                e_tiles[kb] = e4
            else:
                e4 = e_tiles[kb]
            s_ps = spool.tile([P, 1024], F32, tag="s", name="s_ps")
            cc, hh = h // 2, h % 2
            base = 64 * hh
            if w == 640:
                pieces = [(0, 0, 320), (320, 512, 320)]
            else:
                pieces = [(0, 0, w)]
            lhsT = kT8[base:base + 64, cc, :, kb * P:(kb + 1) * P]
            for qoff, poff, pw in pieces:
                rhs = (qT8[base:base + 64, cc, qlo + qoff:qlo + qoff + pw]
                       .unsqueeze(1).broadcast_to([64, 2, pw]))
                nc.tensor.matmul(
                    s_ps[:, poff:poff + pw],
                    lhsT=lhsT, rhs=rhs, start=True, stop=True,
                    perf_mode=DR, skip_group_check=True,
                )
            if w == 640:
                src = s_ps[:].rearrange("p (g c) -> p g c", g=2)[:, :, 0:320]
                dst = e4[:, h, :].rearrange("p (g c) -> p g c", g=2)
            else:
                src = s_ps[:, 0:w]
                dst = e4[:, h, 0:w]
            nc.scalar.activation(
                dst, src, mybir.ActivationFunctionType.Exp, scale=EXP_SCALE)

        def masks(kb, pair, e_tiles):
            e4 = e_tiles[kb]
            qlo, w = _window(kb)
            hs = slice(2 * pair, 2 * pair + 2)
            if kb >= 2:
                nc.gpsimd.affine_select(
                    out=e4[:, hs, 0:P], in_=e4[:, hs, 0:P],
                    compare_op=mybir.AluOpType.is_ge, fill=0.0,
                    base=0, pattern=[[0, 2], [1, P]], channel_multiplier=-1,
                )
            if kb <= KB - 3:
                nc.gpsimd.affine_select(
                    out=e4[:, hs, w - P:w], in_=e4[:, hs, w - P:w],
                    compare_op=mybir.AluOpType.is_ge, fill=0.0,
                    base=0, pattern=[[0, 2], [-1, P]], channel_multiplier=1,
                )

        def phase_c(pair, qt, e_tiles):
            """AV (pair in one bank), recip + stride-0 normalize, fp16
            transpose, dithered fp8 oT write."""
            kbs = range(max(0, qt - 2), min(KB, qt + 3))
            ot = otpool.tile([P, 512], F32, tag="ot", name="ot")
            for hh in range(2):
                h = 2 * pair + hh
                for i, kb in enumerate(kbs):
                    qlo, _ = _window(kb)
                    off = qt * P - qlo
                    nc.tensor.matmul(
                        ot[:, hh * 65:hh * 65 + 65],
                        lhsT=e_tiles[kb][:, h, off:off + P],
                        rhs=v_ext[:, kb, h * 65:h * 65 + 65],
                        start=(i == 0), stop=(i == len(kbs) - 1),
                        skip_group_check=True,
                    )
            o_pair = ot[:, 0:130].rearrange("p (h c) -> p h c", h=2)
            rr = rr_pool.tile([P, 2], F32, tag="rr", name="rr")
            nc.vector.reciprocal(rr[:], o_pair[:, :, HD:HD + 1])
            opr = opr_pool.tile([P, P], F16, tag="opr", name="opr")
            nc.vector.tensor_mul(
                opr[:].rearrange("p (h c) -> p h c", h=2),
                o_pair[:, :, 0:HD],
                rr[:].rearrange("p (h o) -> p h o", h=2).broadcast_to([P, 2, HD]),
            )
            t_ps = ot[:, 256:320].bitcast(F16)
            nc.tensor.transpose(t_ps, opr[:], ident[:])
            nc.vector.tensor_copy(oT[:, pair, qt * P:(qt + 1) * P], t_ps)

        def phase_d(qt, step):
            y_sb = ysb_pool.tile([P, D], F16, tag="ysb", name="y_sb")
            tail = qt >= KB - 2
            late = step >= 11
            for half in range(2):
                y_ps = pqpool.tile([P, 512], F32, tag="pq", name="y_ps")
                hsl = slice(half * 512, (half + 1) * 512)
                for cc in range(2):
                    nc.tensor.matmul(
                        y_ps[:],
                        lhsT=oT[:, cc, qt * P:(qt + 1) * P],
                        rhs=wo_sb[:, cc, hsl],
                        start=(cc == 0), stop=(cc == 1),
                    )
                if tail:
                    # quarter copies on alternating engines + half DMAs
                    q0 = slice(half * 512, half * 512 + 256)
                    q1 = slice(half * 512 + 256, half * 512 + 512)
                    nc.vector.tensor_copy(y_sb[:, q0], y_ps[:, 0:256])
                    nc.scalar.copy(y_sb[:, q1], y_ps[:, 256:512])
                    nc.sync.dma_start(y_d[qt * P:(qt + 1) * P, hsl],
                                      y_sb[:, hsl])
                elif late or half == 0:
                    nc.vector.tensor_copy(y_sb[:, hsl], y_ps[:])
                else:
                    nc.scalar.copy(y_sb[:, hsl], y_ps[:])
            if not tail:
                nc.sync.dma_start(y_d[qt * P:(qt + 1) * P, :], y_sb[:])

        # ---- schedule ---------------------------------------------------
        proj_a, proj_b = {}, {}
        proj_a[1] = [lambda: qk_chain(0, 1), lambda: qk_chain(2, 1)]
        proj_b[1] = [lambda: qk_chain(1, 1), lambda: qk_chain(3, 1)]
        proj_a[3] = [lambda: qk_chain(0, 2)]
        proj_b[3] = [lambda: qk_chain(2, 2)]
        proj_a[4] = [lambda: qk_chain(1, 2)]
        proj_b[4] = [lambda: qk_chain(3, 2)]
        proj_a[6] = [lambda: qk_chain(0, 3)]
        proj_b[6] = [lambda: qk_chain(2, 3)]
        proj_a[7] = [lambda: qk_chain(1, 3)]
        proj_b[7] = [lambda: qk_chain(3, 3)]
        # v blocks: emitted in the early slot (before phase_c of the
        # step), respecting wv DMA arrival (~step 2)
        v_sched_a = {3: [2], 4: [3, 4]}
        for s in range(5, KB):
            v_sched_a[s] = [s]
        v_sched_b = {2: [0, 1]}

        # prologue: wave 0 chains
        for fc in (0, 2, 1, 3):
            qk_chain(fc, 0)

        e_tiles = {}
        for step in range(KB + C1_LAG):
            kb = step if step < KB else None
            qt = step - C1_LAG
            qt2 = step - D_LAG
            if kb is not None:
                phase_b(0, kb, e_tiles)
                phase_b(1, kb, e_tiles)
                masks(kb, 0, e_tiles)
            for f in proj_a.get(step, ()):
                f()
            for n in v_sched_a.get(step, ()):
                v_chain(n)
            if qt in range(KB):
                phase_c(0, qt, e_tiles)
            if kb is not None:
                phase_b(2, kb, e_tiles)
                phase_b(3, kb, e_tiles)
                masks(kb, 1, e_tiles)
            for f in proj_b.get(step, ()):
                f()
            for n in v_sched_b.get(step, ()):
                v_chain(n)
            if qt in range(KB):
                phase_c(1, qt, e_tiles)
            if qt2 in range(KB):
                phase_d(qt2, step)
            if qt == KB - 1:
                phase_d(KB - 1, step)


_NC_CACHE = None


def _get_nc():
    global _NC_CACHE
    if _NC_CACHE is None:
        _NC_CACHE = _build_nc()
    return _NC_CACHE


def _fp8_split(a):
    hi = a.astype(E4M3)
    lo = (a - hi.astype(np.float32)).astype(E4M3)
    return hi, lo


def kernel(x, qkv_w, qkv_b, out_w, out_b):
    x = np.asarray(x, dtype=np.float32)
    qkv_w = np.asarray(qkv_w, dtype=np.float32)
    qkv_b = np.asarray(qkv_b, dtype=np.float32)
    out_w = np.asarray(out_w, dtype=np.float32)
    out_b = np.asarray(out_b, dtype=np.float32)
    B = x.shape[0]
    assert x.shape == (B, L, D) and B * 4 == N_CORES

    nc = _get_nc()

    xs = []
    for b in range(B):
        xt = np.ascontiguousarray(x[b].T)            # [D, L]
        x8, xe8 = _fp8_split(xt)
        xs.append((
            np.ascontiguousarray(x8.reshape(8, P, L).transpose(1, 0, 2)),
            np.ascontiguousarray(xe8.reshape(8, P, L).transpose(1, 0, 2)),
        ))

    in_maps = []
    for core in range(N_CORES):
        b, g = divmod(core, 4)
        rq = slice(g * FV, (g + 1) * FV)
        rk = slice(D + g * FV, D + (g + 1) * FV)
        rv = slice(2 * D + g * FV, 2 * D + (g + 1) * FV)
        wqk_t = np.concatenate([qkv_w[rq], qkv_w[rk]], axis=0).T * WS
        w8, we8 = _fp8_split(wqk_t)

        def to_fc(a):
            return np.ascontiguousarray(
                a.reshape(D, 4, P).transpose(1, 0, 2)
                .reshape(4, 8, P, P).transpose(2, 0, 1, 3))

        wv_t = qkv_w[rv].T * WS                       # [D, 256]
        wv8, wve8 = _fp8_split(wv_t)

        def to_v(a):
            return np.ascontiguousarray(a.reshape(8, P, FV).transpose(1, 0, 2))

        wo16 = out_w[:, g * FV:(g + 1) * FV].T.astype(np.float16)  # [256, D]
        wo16 = np.ascontiguousarray(wo16.reshape(2, P, D).transpose(1, 0, 2))

        bqk = np.ascontiguousarray(
            (WS * np.concatenate([qkv_b[rq], qkv_b[rk]])).reshape(4, P).T)
        bv = np.ascontiguousarray((WS * qkv_b[rv]).reshape(1, FV))
        in_maps.append({
            "x8": xs[b][0], "xe8": xs[b][1],
            "wqk8": to_fc(w8), "wqke8": to_fc(we8),
            "wv8": to_v(wv8), "wve8": to_v(wve8),
            "wo16": wo16,
            "bqk": bqk, "bv": bv,
        })

    res = run_bass_kernel_spmd(nc, in_maps, list(range(N_CORES)))
    y = np.empty((B, L, D), dtype=np.float32)
    for b in range(B):
        acc = res.results[b * 4 + 0]["y"].astype(np.float32)
        for g in range(1, 4):
            acc = acc + res.results[b * 4 + g]["y"]
        y[b] = acc / WS
    if np.any(out_b):
        y += out_b
    return y
